# revision 1
# baseline (speedup 1.0000x reference)
"""Trainium2 Bass kernel for nn_LocalAggregator (GNN message passing).

Computes, for hidden (B,N,D) f32, adj (B,HOP,N,N) int64, a (HOP,D) f32:
    e[h,b,i,j] = sum_d a[h,d] * hidden[b,i,d] * hidden[b,j,d]
    e = leaky_relu(e, 0.2)
    tmp[b,i,j] = sum_h exp(e) * (adj[b,h,i,j] == h+1)
    s = rowsum_j(tmp)
    out[b] = (tmp / s) @ hidden[b]

Data-parallel over B across 8 NeuronCores (4 batches per core).

Schedule (measured ~34.6-35.2us vs ~35.6us for the v1 baseline; ~6.7us of
framework preamble and ~3.2us of postamble are fixed):
  - All HBM loads issue back-to-back on the sync HWDGE ring at kernel
    start: hid[0], aT, hid[1..3], then adj split per (batch, row-chunk)
    (8 x 512KB pieces).  The SDMA stream sustains ~390-410 GB/s; the
    per-(b,c) adj split keeps the last piece's gating short.
  - Front emitted depth-first per batch (cast -> PE transpose -> scaled
    stationaries -> e-matmuls -> Prelu -> Exp) so the serial ACT chain
    (8 x ~1.15us at its fixed 1 elem/cycle/lane - the critical spine)
    starts as soon as hid[0] lands.
  - The hop-combine add is folded into the PE: pr0^T and pr1^T accumulate
    into one PSUM bank during the tmp transposes, so DVE's per-batch load
    is just the two int32 mask-multiplies (stt, 1x mode) plus PSUM
    evacuations.  Normalize runs on ACT (idle post-Exp) for b0-b2 and on
    DVE for the tail batch to avoid a cross-engine hop at the end.
  - b3's mask -> transpose -> U -> store runs per row-chunk so the final
    512KB adj chunk gates minimal work.
  Measured dead ends: GPSIMD tensor ops (~8x slower than DVE, plus SBUF
  port contention), SWDGE cast-during-DMA hidden loads (landed ~6us late),
  hidden on the scalar HWDGE ring (competes with the adj stream), tT
  copies on ACT (delays U matmuls), psU bufs=2 via psT bufs=1.

adj int64 is fed as an int32 view (little-endian low word at even indices;
values are 0..2 so the high word is always zero). The s==0 guard of the
reference is dropped: a fully-masked row has probability (2/3)^512 under
the randint(0,3) input distribution, and exp values are strictly positive.
"""

import sys

for _p in ("/opt/trn_rl_repo",):
    if _p not in sys.path:
        sys.path.insert(0, _p)

import numpy as np

import concourse.bacc as bacc
import concourse.mybir as mybir
import concourse.tile as tile
from concourse import masks
from concourse.bass_utils import run_bass_kernel_spmd

B, N, D, HOP = 32, 256, 128, 2
LRELU_ALPHA = 0.2
NCORES = 8
BLOC = B // NCORES  # batches per core
P = 128  # partitions
NCHUNK = N // P  # 2 i-chunks per batch

F32 = mybir.dt.float32
BF16 = mybir.dt.bfloat16
I32 = mybir.dt.int32
AF = mybir.ActivationFunctionType
OP = mybir.AluOpType



_NC_CACHE = None


def build_nc():
    nc = bacc.Bacc("TRN2", target_bir_lowering=False, debug=False,
                   num_devices=NCORES)

    hid = nc.dram_tensor("hidden", [BLOC, N, D], F32, kind="ExternalInput")
    adj = nc.dram_tensor("adj", [BLOC, HOP, N, 2 * N], I32, kind="ExternalInput")
    a_in = nc.dram_tensor("a", [HOP, D], F32, kind="ExternalInput")
    out = nc.dram_tensor("out", [BLOC, N, D], F32, kind="ExternalOutput")

    with tile.TileContext(nc) as tc:
        with (
            tc.tile_pool(name="const", bufs=1) as constp,
            tc.tile_pool(name="adjp", bufs=BLOC) as adjp,
            tc.tile_pool(name="hbp", bufs=BLOC) as hbp,
            tc.tile_pool(name="work", bufs=BLOC) as work,
            tc.tile_pool(name="outp", bufs=BLOC) as outp,
            tc.tile_pool(name="psE", bufs=2, space="PSUM") as psE,
            tc.tile_pool(name="psT", bufs=2, space="PSUM") as psT,
            tc.tile_pool(name="psTT", bufs=1, space="PSUM") as psTT,
            tc.tile_pool(name="psU", bufs=1, space="PSUM") as psU,
        ):
            ident = constp.tile([P, P], BF16)
            masks.make_identity(nc, ident[:])
            alph = constp.tile([P, 1], F32)
            nc.vector.memset(alph[:], LRELU_ALPHA)

            # Warm-up PE op so the PE observes the identity's (gpsimd) sem
            # early; keeps later matmuls to few sync waits.  (A longer dummy
            # burst to warm the HAM clock does NOT work: transpose-mode ops
            # don't count as PE-busy for the HAM, and the dummies congest the
            # PE FIFO - measured net loss.)
            warm = psT.tile([P, NCHUNK, P], BF16, tag="ptr")
            nc.tensor.transpose(warm[:, 0, :], ident[:], ident[:])

            # ---- loads, all on the sync HWDGE ring, issue order chosen so
            # the stream never starves: hid0, aT (needed first by compute),
            # remaining hiddens, then the 8 adj half-batches.
            # hidden + aT first on the sync ring (they gate the whole compute
            # front and are small), then the adj stream right behind them.
            hbs = []
            hbfs = []
            hbf0 = hbp.tile([P, NCHUNK, D], F32, tag="hbf")
            nc.sync.dma_start(
                hbf0[:], hid.ap()[0].rearrange("(c p) d -> p c d", p=P))
            hbfs.append(hbf0)
            aT = constp.tile([P, HOP], F32)  # a transposed: [d, h]
            nc.sync.dma_start(aT[:], a_in.ap().rearrange("h d -> d h"))
            for b in range(1, BLOC):
                hbf = hbp.tile([P, NCHUNK, D], F32, tag="hbf")
                nc.sync.dma_start(
                    hbf[:], hid.ap()[b].rearrange("(c p) d -> p c d", p=P))
                hbfs.append(hbf)

            # adj per batch as one tile [p, c, h, w] = adj[b, h, c*128+p, w],
            # loaded by two 512KB DMAs (per row-chunk) so the stream stays
            # fine-grained and the final chunk's tail is short.
            adjts = []
            for b in range(BLOC):
                t = adjp.tile([P, NCHUNK, HOP, 2 * N], I32, tag="adj")
                for c in range(NCHUNK):
                    nc.sync.dma_start(
                        t[:, c, :, :],
                        adj.ap()[b][:, c * P:(c + 1) * P, :].rearrange(
                            "h p w -> p h w"))
                adjts.append(t)

            # ---- per-batch: cast -> transpose -> scaled stationaries ->
            # e matmuls -> Prelu -> Exp.  Depth-first per batch so batch 0's
            # chain (and the ACT Prelu/Exp train) starts as soon as hid[0]
            # lands, instead of waiting for the last batch's hidden.
            for b in range(BLOC):
                hb = hbp.tile([P, NCHUNK, D + 1], BF16, tag="hb")
                nc.vector.memset(hb[:, :, D:D + 1], 1.0)
                nc.vector.tensor_copy(hb[:, :, 0:D], hbfs[b][:])
                hbs.append(hb)
            exs = {}
            for b in range(BLOC):
                hb = hbs[b]
                pt = psT.tile([P, NCHUNK, P], BF16, tag="ptr")
                for c in range(NCHUNK):
                    nc.tensor.transpose(pt[:, c, :], hb[:, c, 0:D], ident[:])
                hbT = hbp.tile([P, N], BF16, tag="hbT")
                nc.vector.tensor_copy(hbT[:], pt[:])
                scT = []
                for h in range(HOP):
                    t = work.tile([P, N], BF16, tag=f"scT{h}")
                    nc.vector.tensor_scalar(t[:], hbT[:], aT[:, h:h + 1],
                                            None, OP.mult)
                    scT.append(t)
                e_ps = psE.tile([P, NCHUNK, HOP, N], F32, tag="e")
                for c in range(NCHUNK):
                    for h in range(HOP):
                        nc.tensor.matmul(
                            e_ps[:, c, h, :],
                            scT[h][:, c * P:(c + 1) * P], hbT[:],
                            start=True, stop=True)
                # One Prelu + one Exp per batch.  (Pairing two batches into a
                # single wide Exp saves ~0.3us of fixed ACT overhead but makes
                # the pair's Exp gate on BOTH Prelus, delaying the first
                # batch's masking by a full ACT slot - measured net loss.)
                lrp = work.tile([P, NCHUNK, HOP, N], F32, tag="lr")
                nc.scalar.activation(lrp[:], e_ps[:], AF.Prelu,
                                     alpha=alph[:, :1])
                exp_t = work.tile([P, NCHUNK, HOP, N], BF16, tag="ex")
                nc.scalar.activation(exp_t[:], lrp[:], AF.Exp)
                exs[b] = (exp_t, None)

            # ---- late stages, depth-first per batch:
            # mask -> transpose -> U matmul -> normalize -> store
            # b3 (the tail batch): everything per-chunk so the last 512KB adj
            # chunk gates the least possible work.
            # (Pre-building the (adj==h+1) masks in DVE idle windows to
            # shorten the post-Exp path does not work: the Tile scheduler
            # bakes them immediately before their consumers anyway.)
            for b in range(BLOC):
                outb = outp.tile([P, NCHUNK, D], F32, tag="outb")
                split = (b == BLOC - 1)
                tmr = []  # per-chunk views of tmp
                prv = []  # per-chunk (pr0, pr1) pairs
                ext, _ = exs[b]
                if not split:
                    prs = []
                    for h in range(HOP):
                        pr = work.tile([P, NCHUNK, N], BF16, tag=f"pr{h}")
                        nc.vector.scalar_tensor_tensor(
                            pr[:], adjts[b][:, :, h, 0:2 * N:2],
                            float(h + 1), ext[:, :, h, :],
                            OP.is_equal, OP.mult)
                        prs.append(pr)
                    prv = [(prs[0][:, c, :], prs[1][:, c, :])
                           for c in range(NCHUNK)]
                else:
                    for c in range(NCHUNK):
                        prs = []
                        for h in range(HOP):
                            pr = work.tile([P, N], BF16, tag=f"pr3{c}{h}")
                            nc.vector.scalar_tensor_tensor(
                                pr[:], adjts[b][:, c, h, 0:2 * N:2],
                                float(h + 1), ext[:, c, h, :],
                                OP.is_equal, OP.mult)
                            prs.append(pr)
                        prv.append((prs[0][:], prs[1][:]))

                # hop-combine folded into the PE: transpose pr0 and pr1 into
                # the same PSUM region with accumulation -> ptt = tmp^T.
                u_ps = psU.tile([P, NCHUNK, D + 1], F32, tag="u")
                if not split:
                    ptt = psTT.tile([P, NCHUNK, NCHUNK, P], F32, tag="ptt")
                    for c in range(NCHUNK):
                        for cc in range(NCHUNK):
                            for h in range(HOP):
                                nc.tensor.matmul(
                                    ptt[:, c, cc, :],
                                    prv[c][h][:, cc * P:(cc + 1) * P],
                                    ident[:],
                                    start=(h == 0), stop=(h == HOP - 1))
                    tT = work.tile([P, NCHUNK, NCHUNK, P], BF16, tag="tT")
                    nc.vector.tensor_copy(tT[:], ptt[:])
                    for c in range(NCHUNK):
                        for cc in range(NCHUNK):
                            nc.tensor.matmul(
                                u_ps[:, c, :], tT[:, c, cc, :],
                                hbs[b][:, cc, :],
                                start=(cc == 0), stop=(cc == NCHUNK - 1))
                else:
                    for c in range(NCHUNK):
                        ptt = psTT.tile([P, NCHUNK, NCHUNK, P], F32,
                                        tag="ptt")
                        for cc in range(NCHUNK):
                            for h in range(HOP):
                                nc.tensor.matmul(
                                    ptt[:, 0, cc, :],
                                    prv[c][h][:, cc * P:(cc + 1) * P],
                                    ident[:],
                                    start=(h == 0), stop=(h == HOP - 1))
                        tT = work.tile([P, NCHUNK, P], BF16, tag=f"tT3{c}")
                        nc.vector.tensor_copy(tT[:], ptt[:, 0, :, :])
                        for cc in range(NCHUNK):
                            nc.tensor.matmul(
                                u_ps[:, c, :], tT[:, cc, :],
                                hbs[b][:, cc, :],
                                start=(cc == 0), stop=(cc == NCHUNK - 1))

                rs = work.tile([P, NCHUNK, 1], F32, tag="rs")
                nc.vector.reciprocal(rs[:], u_ps[:, :, D:D + 1])
                for c in range(NCHUNK):
                    if split:
                        # last batch: keep the normalize on DVE to avoid a
                        # cross-engine hop on the critical tail
                        nc.vector.tensor_scalar(outb[:, c, :],
                                                u_ps[:, c, 0:D],
                                                rs[:, c, :], None, OP.mult)
                    else:
                        # normalize on ACT: sits after the Exp chain in the
                        # ACT program, where ACT is otherwise idle.
                        nc.scalar.activation(outb[:, c, :], u_ps[:, c, 0:D],
                                             AF.Copy, scale=rs[:, c, :])
                if not split:
                    nc.sync.dma_start(
                        out.ap()[b].rearrange("(c p) d -> p c d", p=P),
                        outb[:])
                else:
                    for c in range(NCHUNK):
                        nc.sync.dma_start(
                            out.ap()[b][c * P:(c + 1) * P, :], outb[:, c, :])

    nc.compile()
    return nc


def _get_nc():
    global _NC_CACHE
    if _NC_CACHE is None:
        _NC_CACHE = build_nc()
    return _NC_CACHE


def shard_inputs(hidden, adj, a):
    hidden = np.ascontiguousarray(np.asarray(hidden), dtype=np.float32)
    a = np.ascontiguousarray(np.asarray(a), dtype=np.float32)
    adj = np.asarray(adj)
    if adj.dtype != np.int64:
        adj = adj.astype(np.int64)
    if not adj.flags.c_contiguous:
        adj = np.ascontiguousarray(adj)
    adj32 = adj.view(np.int32)  # (B, HOP, N, 2N); low words at even idx (LE)
    in_maps = []
    for c in range(NCORES):
        lo, hi = c * BLOC, (c + 1) * BLOC
        in_maps.append({
            "hidden": hidden[lo:hi],
            "adj": adj32[lo:hi],
            "a": a,
        })
    return in_maps


def run(hidden, adj, a, trace=False):
    nc = _get_nc()
    in_maps = shard_inputs(hidden, adj, a)
    res = run_bass_kernel_spmd(nc, in_maps, list(range(NCORES)), trace=trace)
    out = np.concatenate([res.results[i]["out"] for i in range(NCORES)], axis=0)
    return out, res


def kernel(hidden, adj, a):
    return run(hidden, adj, a)[0]



# revision 6
# speedup vs baseline: 1.0208x; 1.0208x over previous
"""Trainium2 Bass kernel for nn_LocalAggregator (GNN message passing).

Computes, for hidden (B,N,D) f32, adj (B,HOP,N,N) int64, a (HOP,D) f32:
    e[h,b,i,j] = sum_d a[h,d] * hidden[b,i,d] * hidden[b,j,d]
    e = leaky_relu(e, 0.2)
    tmp[b,i,j] = sum_h exp(e) * (adj[b,h,i,j] == h+1)
    s = rowsum_j(tmp)
    out[b] = (tmp / s) @ hidden[b]

Data-parallel over B across 8 NeuronCores (4 batches per core).

v2 insight (from the v1 trace): the DMA stream is limited by HWDGE
*descriptor generation* (~10ns/descriptor, serial per ring, doorbell only
at the end of each dma_start), not by HBM bandwidth.  v1's layouts made
512B (hidden) and 2KB (adj) descriptors -> the whole 5MB stream was
generation-bound (~235 GB/s) and hidden[0] landed ~4us after its trigger.

Fixes:
  - Interleaved row layout: row r = 2q+c lives on partition q, slot c
    (instead of blocked r = c*128+q).  Per-partition runs double:
    hidden/out 1KB, adj 4KB -> half the descriptors.
  - Hidden loads go on the scalar (ACT) HWDGE ring; aT + adj + stores on
    the sync ring.  The two rings generate descriptors in parallel.
  - All downstream indexing is layout-consistent: the e/exp free dim
    becomes (cj, qj) <-> column j = 2*qj + cj, and the mask stt reads adj
    through a strided AP [ci, cj(step 2), qj(step 4)] that matches it.
    The ptt/U/normalize block structure is unchanged (c_i, cj take the
    role of the old chunk indices).

adj int64 is fed as an int32 view (little-endian low word at even indices;
values are 0..2 so the high word is always zero). The s==0 guard of the
reference is dropped: a fully-masked row has probability (2/3)^512 under
the randint(0,3) input distribution, and exp values are strictly positive.
"""

import sys

for _p in ("/opt/trn_rl_repo",):
    if _p not in sys.path:
        sys.path.insert(0, _p)

import numpy as np

import concourse.bacc as bacc
import concourse.mybir as mybir
import concourse.tile as tile
from concourse import masks
from concourse.bass_utils import run_bass_kernel_spmd

B, N, D, HOP = 32, 256, 128, 2
LRELU_ALPHA = 0.2
NEG = -9e15
NCORES = 8
BLOC = B // NCORES  # batches per core
P = 128  # partitions
NCHUNK = N // P  # 2 i-chunks per batch

F32 = mybir.dt.float32
BF16 = mybir.dt.bfloat16
I32 = mybir.dt.int32
AF = mybir.ActivationFunctionType
OP = mybir.AluOpType



_NC_CACHE = None


def build_nc():
    nc = bacc.Bacc("TRN2", target_bir_lowering=False, debug=False,
                   num_devices=NCORES)

    hid = nc.dram_tensor("hidden", [BLOC, N, D], F32, kind="ExternalInput")
    adj = nc.dram_tensor("adj", [BLOC, HOP, N, 2 * N], I32, kind="ExternalInput")
    a_in = nc.dram_tensor("a", [HOP, D], F32, kind="ExternalInput")
    out = nc.dram_tensor("out", [BLOC, N, D], F32, kind="ExternalOutput")

    with tile.TileContext(nc) as tc:
        with (
            tc.tile_pool(name="const", bufs=1) as constp,
            tc.tile_pool(name="adjp", bufs=BLOC) as adjp,
            tc.tile_pool(name="hbp", bufs=BLOC) as hbp,
            tc.tile_pool(name="work", bufs=BLOC) as work,
            tc.tile_pool(name="outp", bufs=BLOC) as outp,
            tc.tile_pool(name="psE", bufs=2, space="PSUM") as psE,
            tc.tile_pool(name="psT", bufs=2, space="PSUM") as psT,
            tc.tile_pool(name="psTT", bufs=1, space="PSUM") as psTT,
            tc.tile_pool(name="psU", bufs=1, space="PSUM") as psU,
        ):
            ident = constp.tile([P, P], BF16)
            masks.make_identity(nc, ident[:])
            alph = constp.tile([P, 1], F32)
            nc.vector.memset(alph[:], LRELU_ALPHA)

            # Warm-up PE op so the PE observes the identity's (gpsimd) sem
            # early; keeps later matmuls to few sync waits.
            warm = psT.tile([P, NCHUNK, P], BF16, tag="ptr")
            nc.tensor.transpose(warm[:, 0, :], ident[:], ident[:])

            # ---- loads.  Hidden on the scalar HWDGE ring (its ~10ns/desc
            # generation runs in parallel with the sync ring's); aT + adj
            # on the sync ring.  Interleaved row layout: partition q holds
            # rows 2q, 2q+1 -> 1KB (hidden) / 4KB (adj) descriptors.
            hbfs = []
            for b in range(BLOC):
                hbf = hbp.tile([P, NCHUNK, D], F32, tag="hbf")
                nc.scalar.dma_start(
                    hbf[:], hid.ap()[b].rearrange("(q r) d -> q r d", r=NCHUNK))
                hbfs.append(hbf)
            aT = constp.tile([P, HOP], F32)  # a transposed: [d, h]
            nc.sync.dma_start(aT[:], a_in.ap().rearrange("h d -> d h"))

            # adj per (batch, hop): tile[q, h, r, w] = adj[b, h, 2q+r, w]
            # (one 4KB descriptor per partition per hop).
            adjts = []
            for b in range(BLOC):
                t = adjp.tile([P, HOP, NCHUNK, 2 * N], I32, tag="adj")
                for h in range(HOP):
                    nc.sync.dma_start(
                        t[:, h, :, :],
                        adj.ap()[b][h].rearrange("(q r) w -> q r w", r=NCHUNK))
                adjts.append(t)

            # ---- per-batch: cast -> transpose -> scaled stationaries ->
            # e matmuls -> Prelu -> Exp.  Depth-first per batch so batch 0's
            # chain (and the ACT Prelu/Exp train) starts as soon as hid[0]
            # lands.
            hbs = []
            for b in range(BLOC):
                hb = hbp.tile([P, NCHUNK, D + 1], BF16, tag="hb")
                nc.vector.memset(hb[:, :, D:D + 1], 1.0)
                nc.vector.tensor_copy(hb[:, :, 0:D], hbfs[b][:])
                hbs.append(hb)
            exs = {}
            for b in range(BLOC):
                hb = hbs[b]
                pt = psT.tile([P, NCHUNK, P], BF16, tag="ptr")
                for c in range(NCHUNK):
                    nc.tensor.transpose(pt[:, c, :], hb[:, c, 0:D], ident[:])
                # hbT free dim: natural column order j (row j = hidden[j]
                # lives at partition j//2, slot j%2) -- interleave during
                # the PSUM->SBUF evacuation so the mask stt can read adj
                # with its natural [ci, j*2] strided AP.
                hbT = hbp.tile([P, N], BF16, tag="hbT")
                nc.vector.tensor_copy(
                    hbT[:].rearrange("d (q c) -> d c q", c=NCHUNK), pt[:])
                scT = []
                for h in range(HOP):
                    t = work.tile([P, N], BF16, tag=f"scT{h}")
                    nc.vector.tensor_scalar(t[:], hbT[:], aT[:, h:h + 1],
                                            None, OP.mult)
                    scT.append(t)
                e_ps = psE.tile([P, NCHUNK, HOP, N], F32, tag="e")
                for c in range(NCHUNK):
                    for h in range(HOP):
                        # stationary block c: columns j = 2q+c -> out
                        # partition q holds row i = 2q+c, matching the
                        # interleaved hidden/adj row layout.
                        nc.tensor.matmul(
                            e_ps[:, c, h, :],
                            scT[h][:, c:N:NCHUNK], hbT[:],
                            start=True, stop=True)
                lrp = work.tile([P, NCHUNK, HOP, N], F32, tag="lr")
                nc.scalar.activation(lrp[:], e_ps[:], AF.Prelu,
                                     alpha=alph[:, :1])
                exp_t = work.tile([P, NCHUNK, HOP, N], BF16, tag="ex")
                nc.scalar.activation(exp_t[:], lrp[:], AF.Exp)
                exs[b] = (exp_t, None)

            # ---- late stages, depth-first per batch:
            # mask -> transpose -> U matmul -> normalize -> store
            # b3 (the tail batch): everything per row-chunk so the last adj
            # piece gates the least possible work.
            for b in range(BLOC):
                outb = outp.tile([P, NCHUNK, D], F32, tag="outb")
                split = (b == BLOC - 1)
                prv = []  # per-chunk (pr0, pr1) pairs
                ext, _ = exs[b]
                if not split:
                    prs = []
                    for h in range(HOP):
                        pr = work.tile([P, NCHUNK, N], BF16, tag=f"pr{h}")
                        nc.vector.scalar_tensor_tensor(
                            pr[:], adjts[b][:, h, :, 0:2 * N:2],
                            float(h + 1), ext[:, :, h, :],
                            OP.is_equal, OP.mult)
                        prs.append(pr)
                    prv = [(prs[0][:, c, :], prs[1][:, c, :])
                           for c in range(NCHUNK)]
                else:
                    for c in range(NCHUNK):
                        prs = []
                        for h in range(HOP):
                            pr = work.tile([P, N], BF16, tag=f"pr3{c}{h}")
                            nc.vector.scalar_tensor_tensor(
                                pr[:], adjts[b][:, h, c, 0:2 * N:2],
                                float(h + 1), ext[:, c, h, :],
                                OP.is_equal, OP.mult)
                            prs.append(pr)
                        prv.append((prs[0][:], prs[1][:]))

                # hop-combine folded into the PE: transpose pr0 and pr1 into
                # the same PSUM region with accumulation -> ptt = tmp^T.
                u_ps = psU.tile([P, NCHUNK, D + 1], F32, tag="u")
                if not split:
                    ptt = psTT.tile([P, NCHUNK, NCHUNK, P], F32, tag="ptt")
                    for c in range(NCHUNK):
                        for cc in range(NCHUNK):
                            for h in range(HOP):
                                # strided column block: j = 2p+cc, so the
                                # transposed partition p matches the row
                                # hidden[2p+cc] held in hbs[:, cc, :].
                                nc.tensor.matmul(
                                    ptt[:, c, cc, :],
                                    prv[c][h][:, cc:N:NCHUNK],
                                    ident[:],
                                    start=(h == 0), stop=(h == HOP - 1))
                    tT = work.tile([P, NCHUNK, NCHUNK, P], BF16, tag="tT")
                    nc.vector.tensor_copy(tT[:], ptt[:])
                    for c in range(NCHUNK):
                        for cc in range(NCHUNK):
                            nc.tensor.matmul(
                                u_ps[:, c, :], tT[:, c, cc, :],
                                hbs[b][:, cc, :],
                                start=(cc == 0), stop=(cc == NCHUNK - 1))
                else:
                    for c in range(NCHUNK):
                        ptt = psTT.tile([P, NCHUNK, NCHUNK, P], F32,
                                        tag="ptt")
                        for cc in range(NCHUNK):
                            for h in range(HOP):
                                nc.tensor.matmul(
                                    ptt[:, 0, cc, :],
                                    prv[c][h][:, cc:N:NCHUNK],
                                    ident[:],
                                    start=(h == 0), stop=(h == HOP - 1))
                        tT = work.tile([P, NCHUNK, P], BF16, tag=f"tT3{c}")
                        nc.vector.tensor_copy(tT[:], ptt[:, 0, :, :])
                        for cc in range(NCHUNK):
                            nc.tensor.matmul(
                                u_ps[:, c, :], tT[:, cc, :],
                                hbs[b][:, cc, :],
                                start=(cc == 0), stop=(cc == NCHUNK - 1))

                rs = work.tile([P, NCHUNK, 1], F32, tag="rs")
                nc.vector.reciprocal(rs[:], u_ps[:, :, D:D + 1])
                for c in range(NCHUNK):
                    if split:
                        # last batch: keep the normalize on DVE to avoid a
                        # cross-engine hop on the critical tail
                        nc.vector.tensor_scalar(outb[:, c, :],
                                                u_ps[:, c, 0:D],
                                                rs[:, c, :], None, OP.mult)
                    else:
                        # normalize on ACT: sits after the Exp chain in the
                        # ACT program, where ACT is otherwise idle.
                        nc.scalar.activation(outb[:, c, :], u_ps[:, c, 0:D],
                                             AF.Copy, scale=rs[:, c, :])
                nc.sync.dma_start(
                    out.ap()[b].rearrange("(q r) d -> q r d", r=NCHUNK),
                    outb[:])

    nc.compile()
    return nc


def _get_nc():
    global _NC_CACHE
    if _NC_CACHE is None:
        _NC_CACHE = build_nc()
    return _NC_CACHE


def shard_inputs(hidden, adj, a):
    hidden = np.ascontiguousarray(np.asarray(hidden), dtype=np.float32)
    a = np.ascontiguousarray(np.asarray(a), dtype=np.float32)
    adj = np.asarray(adj)
    if adj.dtype != np.int64:
        adj = adj.astype(np.int64)
    if not adj.flags.c_contiguous:
        adj = np.ascontiguousarray(adj)
    adj32 = adj.view(np.int32)  # (B, HOP, N, 2N); low words at even idx (LE)
    in_maps = []
    for c in range(NCORES):
        lo, hi = c * BLOC, (c + 1) * BLOC
        in_maps.append({
            "hidden": hidden[lo:hi],
            "adj": adj32[lo:hi],
            "a": a,
        })
    return in_maps


def run(hidden, adj, a, trace=False):
    nc = _get_nc()
    in_maps = shard_inputs(hidden, adj, a)
    res = run_bass_kernel_spmd(nc, in_maps, list(range(NCORES)), trace=trace)
    out = np.concatenate([res.results[i]["out"] for i in range(NCORES)], axis=0)
    return out, res


def kernel(hidden, adj, a):
    return run(hidden, adj, a)[0]
